# revision 32
# baseline (speedup 1.0000x reference)
"""Trainium2 Bass kernel for nn_ConvSplitTree.

Math (see reference):
  value = sigmoid(conv2d(x, wnorm))                [N,6,H,W]
  leaf  = sum_d (value_d < 0.6) * 2^(5-d)          [N,H,W]
  pred  = conv2d(data, w_pred) + b_pred            [N,64,H,W]
  y     = sum_c pred_c / 64 + pred[leaf]           [N,H,W]

Sharding: 8 shards = 4 samples x 2 image halves (256 rows each), data
parallel; the 1-row conv halo is handled by host-side zero padding.

Per-core layout (32-row groups, 2-row pred blocks), engine budget per
group roughly PE 18us / DVE 16us / ACT 11us / SP 10us / Pool 7us:
  - pred conv: bf16 matmuls (1 PE cycle/row; fp32 would be 4).
    K=128 = (4 dw x 32 cin) where dw is a 4-row window covering BOTH
    output rows of a 2-row block: out partition (half*64+co); weights
    for half h sit at dw = dy + h.  3 dx-shifted matmuls accumulate
    the SAME-conv in one [128,512] PSUM tile = 2 rows x 64 channels.
    Data is host-replicated into the (dw,ci) layout (data4, bf16) so
    each group loads with two large simple-stride DMAs.
  - split conv: 8-row blocks, K=72 = (9 taps x 8 rows), M=48+16 pad.
    x is host-replicated x9 (row+col shifts, bf16 hi/lo pair).  Three
    bf16 matmuls Wh*xh + Wh*xl + Wl*xh recover ~fp32 accuracy (the
    dropped Wl*xl term is ~2^-18); the leaf bits must not flip.
  - bits = sign(value - THR) in {-1,+1} on the ACT engine; THR is the
    fp32 boundary of sigmoid(v) < 0.6.  Zero-weight pad rows yield a
    constant -1 used as the affine offset row.
  - leaf value broadcast to both 64-channel halves via ONE fp8e4m3
    DoubleRow matmul per block (exact: coefficients are -2^k/-0.5 and
    sign bits, accumulated in fp32 PSUM), 0.5 PE cycles/row.
  - m = (leaf == chan_idx) + 1/64: tensor_scalar on DVE, half the
    blocks offloaded to Pool (gpsimd cannot read PSUM, so those read
    an ACT-staged SBUF copy of leafb).
  - t = (pred + b) * m: one DVE scalar_tensor_tensor op (bf16 out).
  - y(2 rows) = ones.T @ t: one bf16 matmul per block does the 64->1
    channel reduction (base + sel); 4 consecutive blocks land in one
    PSUM bank at col-tile partitions {0,32,64,96}, then one ACT copy
    + one [98,512] DMA per bank writes a DRAM scratch from which the
    host extracts the 8 real rows (host reshuffle is free).
"""

import os
import sys

import ml_dtypes
import numpy as np

for _p in ("/opt/trn_rl_repo", "/root/.axon_site/_ro/trn_rl_repo"):
    if os.path.isdir(_p) and _p not in sys.path:
        sys.path.insert(0, _p)

from contextlib import ExitStack

import concourse.bacc as bacc
import concourse.tile as tile
from concourse import mybir
from concourse.bass_utils import run_bass_kernel_spmd

N_CORES = 8
N, H, W = 4, 512, 512
HC = H // 2          # rows per core
WP = W + 2           # padded width
CIN, COUT, D = 32, 64, 6
RG = 32              # rows per group
NG = HC // RG        # groups per core
NBG = RG // 2        # 2-row blocks per group (16)
NBS = 8              # 2-row blocks per 16-row subgroup
# fp32 boundary: (v < THR) == (float32_sigmoid(v) < 0.6) for all fp32 v
THR = float(np.float32(0.4054651))

F32 = mybir.dt.float32
BF16 = mybir.dt.bfloat16
F8E4 = mybir.dt.float8e4

_PROGRAM = None
LAST_RESULT = None


def _build_program():
    nc = bacc.Bacc(
        "TRN2", target_bir_lowering=False, debug=False, enable_asserts=False
    )
    # host-replicated layouts: data4[dw*32+ci, jg, col] = data_pad[ci, 2*jg+dw, col]
    # and x9[(dy*3+dx)*8+r, bg, col] = x_pad[8*bg + r + dy, col + dx]
    data_d = nc.dram_tensor("data4", [128, HC // 2, WP], BF16, kind="ExternalInput").ap()
    x_dh = nc.dram_tensor("x9h", [72, HC // 8, W], BF16, kind="ExternalInput").ap()
    x_dl = nc.dram_tensor("x9l", [72, HC // 8, W], BF16, kind="ExternalInput").ap()
    wsplit_dh = nc.dram_tensor("wsplit9_th", [72, 64], BF16, kind="ExternalInput").ap()
    wsplit_dl = nc.dram_tensor("wsplit9_tl", [72, 64], BF16, kind="ExternalInput").ap()
    wpred_d = nc.dram_tensor("wpred_t", [128, 3, 128], BF16, kind="ExternalInput").ap()
    leafpow_d = nc.dram_tensor("leafpow", [64, NBS, 2, 128], F8E4, kind="ExternalInput").ap()
    ones2_d = nc.dram_tensor("ones2", [128, 32], BF16, kind="ExternalInput").ap()
    cidx_d = nc.dram_tensor("cidx", [128, 1], F32, kind="ExternalInput").ap()
    bvec_d = nc.dram_tensor("bvec", [128, 1], F32, kind="ExternalInput").ap()
    negthr_d = nc.dram_tensor("negthr", [128, 1], F32, kind="ExternalInput").ap()
    # y_scr[g*4+bg, 32q+h, :] = output row r0 + 8*bg + 2*q + h (other
    # partitions are don't-care; the host extracts the 8 real rows per bank)
    y_d = nc.dram_tensor("y_scr", [NG * 4, 98, W], F32, kind="ExternalOutput").ap()

    eq = mybir.AluOpType.is_equal
    add = mybir.AluOpType.add
    mult = mybir.AluOpType.mult

    with tile.TileContext(nc) as tc, ExitStack() as ctx:
        consts = ctx.enter_context(tc.tile_pool(name="consts", bufs=1))
        s_pool = ctx.enter_context(tc.tile_pool(name="s", bufs=3))
        x_pool = ctx.enter_context(tc.tile_pool(name="x", bufs=3))
        b_pool = ctx.enter_context(tc.tile_pool(name="bits", bufs=3))
        m_pool = ctx.enter_context(tc.tile_pool(name="mwork", bufs=4))
        t_pool = ctx.enter_context(tc.tile_pool(name="twork", bufs=4))
        yc_pool = ctx.enter_context(tc.tile_pool(name="ycop", bufs=2))
        lf_pool = ctx.enter_context(tc.tile_pool(name="lfstage", bufs=2))
        ps_val = ctx.enter_context(tc.tile_pool(name="ps_val", bufs=2, space="PSUM"))
        ps_pred = ctx.enter_context(tc.tile_pool(name="ps_pred", bufs=2, space="PSUM"))
        ps_leaf = ctx.enter_context(tc.tile_pool(name="ps_leaf", bufs=2, space="PSUM"))
        ps_y = ctx.enter_context(tc.tile_pool(name="ps_y", bufs=2, space="PSUM"))

        wsplit_th = consts.tile([72, 64], BF16)
        nc.scalar.dma_start(out=wsplit_th, in_=wsplit_dh)
        wsplit_tl = consts.tile([72, 64], BF16)
        nc.scalar.dma_start(out=wsplit_tl, in_=wsplit_dl)
        wpred_t = consts.tile([128, 3, 128], BF16)
        nc.scalar.dma_start(out=wpred_t, in_=wpred_d)
        leafpow_t = consts.tile([64, NBS, 2, 128], F8E4)
        nc.scalar.dma_start(out=leafpow_t, in_=leafpow_d)
        ones2_t = consts.tile([128, 32], BF16)
        nc.scalar.dma_start(out=ones2_t, in_=ones2_d)
        cidx_t = consts.tile([128, 1], F32)
        nc.scalar.dma_start(out=cidx_t, in_=cidx_d)
        bvec_t = consts.tile([128, 1], F32)
        nc.scalar.dma_start(out=bvec_t, in_=bvec_d)
        negthr_t = consts.tile([128, 1], F32)
        nc.scalar.dma_start(out=negthr_t, in_=negthr_d)

        for g in range(NG):
            r0 = g * RG
            # data stacked by 4-row window: partition (dw*32+ci), slot j
            # holds padded row r0 + 2j + dw.  One DMA per group from the
            # host-replicated data4 layout.
            # x replicated x9: partition (dy*3+dx)*8 + r, slot b
            # (8-row block), col c = x_pad[r0 + 8b + r + dy, c + dx].
            # Issued before S: it is small and the split conv needs it
            # first; the DMA device serves transfers in issue order.
            xTh = x_pool.tile([72, 4, W], BF16, tag="xTh")
            nc.sync.dma_start(out=xTh, in_=x_dh[:, 4 * g : 4 * g + 4, :])
            xTl = x_pool.tile([72, 4, W], BF16, tag="xTl")
            nc.sync.dma_start(out=xTl, in_=x_dl[:, 4 * g : 4 * g + 4, :])

            S = s_pool.tile([128, NBG, WP], BF16, tag="S")
            for hh in range(2):
                h0 = hh * (NBG // 2)
                nc.sync.dma_start(
                    out=S[:, h0 : h0 + NBG // 2, :],
                    in_=data_d[:, g * NBG + h0 : g * NBG + h0 + NBG // 2, :],
                )

            for sub in range(2):
                # split conv: out (r*6+d) per 8 rows, exact fp32
                bits = b_pool.tile([64, 2, W], F8E4, tag="bits")
                for b2 in range(2):
                    b = sub * 2 + b2
                    val8 = ps_val.tile([64, W], F32, tag="val8")
                    for lhsT, rhs, st, sp in (
                        (wsplit_th, xTh, True, False),
                        (wsplit_th, xTl, False, False),
                        (wsplit_tl, xTh, False, True),
                    ):
                        nc.tensor.matmul(
                            val8, lhsT=lhsT, rhs=rhs[:, b, :], start=st, stop=sp
                        )
                    # bits' = sign(val - THR) in {-1,+1}; zero-weight pad
                    # rows give sign(0-THR) = -1, used as the constant row
                    # for the affine leaf map folded into leafpow.
                    nc.scalar.activation(
                        out=bits[:, b2, :],
                        in_=val8,
                        func=mybir.ActivationFunctionType.Sign,
                        bias=negthr_t[0:64, :],
                    )

                for j2 in range(NBS):
                    j = sub * NBS + j2
                    pred = ps_pred.tile([128, W], F32, tag="pred")
                    for dx in range(3):
                        nc.tensor.matmul(
                            pred,
                            lhsT=wpred_t[:, dx, :],
                            rhs=S[:, j, dx : dx + W],
                            start=(dx == 0),
                            stop=(dx == 2),
                        )
                    # leaf broadcast to both 64-partition halves (exact ints)
                    leafb = ps_leaf.tile([128, W], F32, tag="leafb")
                    nc.tensor.matmul(
                        leafb,
                        lhsT=leafpow_t[:, j2, :, :],
                        rhs=bits,
                        start=True,
                        stop=True,
                        perf_mode=mybir.MatmulPerfMode.DoubleRow,
                    )
                    m = m_pool.tile([128, W], BF16, tag="m")
                    if j2 in (1, 3, 5, 7):
                        # GPSIMD cannot read PSUM: stage leafb via ACT copy
                        lfs = lf_pool.tile([128, W], F32, tag="lfs")
                        nc.scalar.copy(lfs, leafb)
                        nc.gpsimd.tensor_scalar(
                            out=m, in0=lfs, scalar1=cidx_t, scalar2=1.0 / 64,
                            op0=eq, op1=add,
                        )
                    else:
                        nc.vector.tensor_scalar(
                            out=m, in0=leafb, scalar1=cidx_t, scalar2=1.0 / 64,
                            op0=eq, op1=add,
                        )
                    # t = (pred + b) * m
                    t = t_pool.tile([128, W], BF16, tag="t")
                    nc.vector.scalar_tensor_tensor(
                        out=t, in0=pred, scalar=bvec_t, in1=m, op0=add, op1=mult
                    )
                    # y matmuls of 4 consecutive blocks land in one PSUM
                    # bank at col-tile partitions 32q; one copy+DMA per bank
                    q = j % 4
                    if q == 0:
                        ybank = ps_y.tile([128, W], F32, tag="ybank")
                    nc.tensor.matmul(
                        ybank[32 * q : 32 * q + 32, :],
                        lhsT=ones2_t,
                        rhs=t,
                        start=True,
                        stop=True,
                        tile_position=(0, 32 * q),
                    )
                    if q == 3:
                        bg = j // 4
                        yc = yc_pool.tile([98, W], F32, tag="yc")
                        nc.scalar.copy(yc, ybank[0:98, :])
                        yeng = (nc.scalar, nc.sync)[bg % 2]
                        yeng.dma_start(out=y_d[g * 4 + bg], in_=yc)
    nc.compile()  # bacc passes (register allocation etc.) before serialization
    return nc


def _consts(w_split, w_pred, b_pred):
    # normalize split weights exactly like the reference (fp32 ops)
    w = np.maximum(w_split.astype(np.float32), np.float32(0.0))
    s = w.sum(axis=(1, 2, 3), keepdims=True, dtype=np.float32)
    wn = np.where(s < np.float32(0.1), w + np.float32(0.1 / 9.0), w)

    # [72, 64]: row (dy*24 + dx*8 + r), col r*6+d (cols 48..63 zero pad)
    wsplit9 = np.zeros((72, 64), np.float32)
    for dy in range(3):
        for dx in range(3):
            for r in range(8):
                for d in range(D):
                    wsplit9[dy * 24 + dx * 8 + r, r * 6 + d] = wn[d, 0, dy, dx]
    wsplit9_h = wsplit9.astype(ml_dtypes.bfloat16)
    wsplit9_l = (wsplit9 - wsplit9_h.astype(np.float32)).astype(ml_dtypes.bfloat16)

    # [co, ci, dy, dx] -> [dw*32+ci, dx, half*64+co], dw = dy + half
    wpred_t = np.zeros((128, 3, 128), np.float32)
    wp = w_pred.astype(np.float32)
    for half in range(2):
        for dy in range(3):
            dw = dy + half
            # [ci, dx, co]
            wpred_t[dw * 32 : dw * 32 + 32, :, half * 64 : half * 64 + 64] = (
                wp[:, :, dy, :].transpose(1, 2, 0)
            )

    pw = (2.0 ** np.arange(5, -1, -1)).astype(np.float32)
    # bits[p, s, :] holds sign(v-THR) in {-1,+1} for subgroup row R = 8s + p//6:
    # leaf = 31.5 - sum_d (pw_d/2)*sign_d.  Pad partitions 48..53 (zero-weight
    # conv rows, sign = -1) carry the +31.5 offset as -{16,8,4,2,1,0.5}.
    # All coefficients are exact in fp8e4m3; DoubleRow halves PE cycles.
    leafpow = np.zeros((64, NBS, 2, 128), np.float32)
    offs = [16.0, 8.0, 4.0, 2.0, 1.0, 0.5]
    for j2 in range(NBS):
        for half in range(2):
            R = 2 * j2 + half
            s = R // 8
            p0 = 6 * (R % 8)
            cols = slice(64 * half, 64 * half + 64)
            for d in range(D):
                leafpow[p0 + d, j2, s, cols] = -pw[d] / 2.0
            for k, v in enumerate(offs):
                leafpow[48 + k, j2, s, cols] = -v

    ones2 = np.zeros((128, 32), np.float32)
    ones2[:64, 0] = 1.0
    ones2[64:, 1] = 1.0
    cidx = (np.arange(128) % 64).astype(np.float32)[:, None]
    bvec = np.concatenate([b_pred, b_pred]).astype(np.float32)[:, None]
    return {
        "wsplit9_th": wsplit9_h,
        "wsplit9_tl": wsplit9_l,
        "wpred_t": wpred_t.astype(ml_dtypes.bfloat16),
        "leafpow": leafpow.astype(ml_dtypes.float8_e4m3),
        "ones2": ones2.astype(ml_dtypes.bfloat16),
        "cidx": cidx,
        "bvec": bvec,
        "negthr": np.full((128, 1), -np.float32(THR), np.float32),
    }


def make_in_maps(x, data, w_split, w_pred, b_pred):
    x = np.asarray(x, np.float32)
    data = np.asarray(data, np.float32)
    consts = _consts(np.asarray(w_split), np.asarray(w_pred), np.asarray(b_pred))

    xp = np.zeros((N, H + 2, WP), np.float32)
    xp[:, 1 : H + 1, 1 : W + 1] = x[:, 0]
    dp = np.zeros((N, CIN, H + 2, WP), np.float32)
    dp[:, :, 1 : H + 1, 1 : W + 1] = data

    in_maps = []
    for c in range(N_CORES):
        n, half = divmod(c, 2)
        r0 = half * HC
        m = dict(consts)
        dpc = dp[n, :, r0 : r0 + HC + 2, :]   # [32, 258, 514]
        xpc = xp[n, r0 : r0 + HC + 2, :]      # [258, 514]
        # data4[dw*32+ci, jg, col] = dpc[ci, 2*jg+dw, col]
        data4 = np.empty((128, HC // 2, WP), np.float32)
        for dw in range(4):
            data4[dw * 32 : dw * 32 + 32] = dpc[:, dw : dw + HC : 2, :]
        # x9[(dy*3+dx)*8+r, bg, col] = xpc[8*bg + r + dy, col + dx]
        x9 = np.empty((9, 8, HC // 8, W), np.float32)
        for dy in range(3):
            for dx in range(3):
                x9[dy * 3 + dx] = (
                    xpc[dy : dy + HC, dx : dx + W]
                    .reshape(HC // 8, 8, W)
                    .transpose(1, 0, 2)
                )
        x9 = x9.reshape(72, HC // 8, W)
        x9h = x9.astype(ml_dtypes.bfloat16)
        x9l = (x9 - x9h.astype(np.float32)).astype(ml_dtypes.bfloat16)
        m["data4"] = data4.astype(ml_dtypes.bfloat16)
        m["x9h"] = x9h
        m["x9l"] = x9l
        in_maps.append(m)
    return in_maps


def kernel(x, data, w_split, w_pred, b_pred):
    global _PROGRAM, LAST_RESULT
    if _PROGRAM is None:
        _PROGRAM = _build_program()
    in_maps = make_in_maps(x, data, w_split, w_pred, b_pred)
    res = run_bass_kernel_spmd(_PROGRAM, in_maps, list(range(N_CORES)))
    LAST_RESULT = res
    y = np.empty((N, H, W), np.float32)
    for c in range(N_CORES):
        n, half = divmod(c, 2)
        scr = res.results[c]["y_scr"]          # [NG*4, 98, W]
        rows = scr[:, [0, 1, 32, 33, 64, 65, 96, 97], :]   # [NG*4, 8, W]
        # bank bg rows are (2q+h) -> ordered [q*2+h] == row offset 8bg+2q+h
        y[n, half * HC : (half + 1) * HC, :] = rows.reshape(HC, W)
    return y
